# revision 13
# baseline (speedup 1.0000x reference)
"""Trainium2 Bass kernel for nn_BaseModel_20925080666480.

Pipeline per core (8 batch rows): 3-layer mean + ragged segment-mean via
one-hot matmul + cap-embedding concat + input projection, then a
segment-parallel BiLSTM (16 segments/dir, 32-step warmup) over 256 words.
"""

import numpy as np

import concourse.bass as bass
import concourse.tile as tile
from concourse import mybir
from concourse.bass_utils import run_bass_kernel_spmd

AF = mybir.ActivationFunctionType
ALU = mybir.AluOpType
F32 = mybir.dt.float32
F16 = mybir.dt.float16
I32 = mybir.dt.int32

NCORES = 8
B = 64
RPC = B // NCORES          # batch rows per core
T, W, D, H = 512, 256, 768, 20
G4 = 4 * H                 # 80 gate rows
GP = 128                   # padded gate rows (32-aligned starts)
GI, GF, GO, GG = 0, 32, 64, 96
CAP = 10
INP_PAD = 896              # 778 padded to 7*128
S = 16                     # segments per direction
EMIT = W // S              # 16 words emitted per segment
WU = 32                    # warmup steps
L = EMIT + WU              # 48 chain steps
HC = S * RPC               # 128 columns per direction
COLS = 2 * HC              # 256 total lstm columns
GIN_F0 = WU * RPC          # fwd zero-pad columns [0, 256)
GIN_B0 = GIN_F0 + W * RPC  # bwd data starts at 2304
GIN_SZ = 2 * (W * RPC + WU * RPC)  # 4608


def _sb_ap(t, offset, dims):
    """Raw AP on a tile's tensor: partition pair from the tile, custom free dims."""
    p = t.ap[0]
    return bass.AP(t.tensor, offset, [[p[0], p[1]], *[list(x) for x in dims]])


def _legalize_waits(nc):
    """This walrus build accepts only ONE sync-wait per instruction; hoist
    extra waits onto NoOps inserted just before, on the same engine."""
    n = 0
    for fn in nc.m.functions:
        for b in fn.blocks:
            insts = b.instructions
            i = 0
            while i < len(insts):
                inst = insts[i]
                si = inst.sync_info
                if si is not None and len(si.on_wait) > 1:
                    waits = list(si.on_wait)
                    for w in waits[:-1]:
                        nop = mybir.InstNoOp(
                            name=f"wnop_{n}", engine=inst.engine, ins=[], outs=[]
                        )
                        nop.sync_info = mybir.SyncInfo(on_wait=[w], on_update=[])
                        insts.insert(i, nop)
                        i += 1
                        n += 1
                    inst.sync_info = mybir.SyncInfo(
                        on_wait=[waits[-1]], on_update=list(si.on_update)
                    )
                i += 1
    return n


def build_nc(legalize=True):
    nc = bass.Bass()
    hid = nc.dram_tensor("hid", [3, RPC, T + 1, D], F32, kind="ExternalInput")
    wihT = nc.dram_tensor("wihT", [INP_PAD, 2 * GP], F16, kind="ExternalInput")
    whhT = nc.dram_tensor("whhT", [H, 2 * GP], F16, kind="ExternalInput")
    bias = nc.dram_tensor("bias", [GP, 2], F32, kind="ExternalInput")
    ids = nc.dram_tensor("ids", [RPC, T], I32, kind="ExternalInput")
    recip3 = nc.dram_tensor("recip3", [RPC, W], F32, kind="ExternalInput")
    capT = nc.dram_tensor("capT", [RPC, CAP, W], F16, kind="ExternalInput")
    # [dir, h, b, w] — host transposes to [b, w, 2H]
    out = nc.dram_tensor("out", [2, H, RPC, W], F32, kind="ExternalOutput")

    with tile.TileContext(nc) as tc:
        with (
            tc.tile_pool(name="const", bufs=1) as cp,
            tc.tile_pool(name="ldp", bufs=3) as ldp,
            tc.tile_pool(name="s3fp", bufs=2) as s3fp,
            tc.tile_pool(name="s3p", bufs=6) as s3p,
            tc.tile_pool(name="Apool", bufs=6) as apool,
            tc.tile_pool(name="xwp", bufs=2) as xwp,
            tc.tile_pool(name="rbp", bufs=2) as rbp,
        ):
            # ---- constants ----
            iota_i = cp.tile([128, W], I32, name="iota_i")
            eng_iota = nc.gpsimd if hasattr(nc.gpsimd, "iota") else nc.vector
            eng_iota.iota(iota_i[:], pattern=[[1, W]], base=0, channel_multiplier=0)
            iota_f = cp.tile([128, W], F32, name="iota_f")
            nc.vector.tensor_copy(iota_f[:], iota_i[:])

            wih_sb = cp.tile([128, 7 * 2 * GP], F16, name="wih_sb")
            nc.sync.dma_start(
                wih_sb[:].rearrange("p (j g) -> p j g", j=7),
                wihT[:].rearrange("(j p) g -> p j g", p=128),
            )
            whh_sb = cp.tile([H, 2 * GP], F16, name="whh_sb")
            nc.sync.dma_start(whh_sb[:], whhT[:])
            bias_sb = cp.tile([GP, 2], F32, name="bias_sb")
            nc.sync.dma_start(bias_sb[:], bias[:])

            ids_sb = cp.tile([128, RPC * 4], I32, name="ids_sb")
            nc.sync.dma_start(
                ids_sb[:].rearrange("p (r k) -> p r k", r=RPC),
                ids[:].rearrange("r (k p) -> p r k", p=128),
            )
            ids_f = cp.tile([128, RPC * 4], F32, name="ids_f")
            nc.vector.tensor_copy(ids_f[:], ids_sb[:])

            recip_sb = cp.tile([1, RPC * W], F32, name="recip_sb")
            nc.sync.dma_start(
                recip_sb[:], recip3[:].rearrange("r w -> (r w)").unsqueeze(0)
            )
            capT_sb = cp.tile([CAP, RPC * W], F16, name="capT_sb")
            nc.sync.dma_start(
                capT_sb[:].rearrange("p (r w) -> p r w", r=RPC),
                capT[:].rearrange("r c w -> c r w"),
            )
            ones_col = cp.tile([1, 128], F32, name="ones_col")
            nc.vector.memset(ones_col[:], 1.0)

            # ---- persistent state ----
            G_in = cp.tile([GP, GIN_SZ], F32, name="G_in")
            nc.vector.memset(G_in[:], 0.0)
            staged_f = cp.tile([H, (L + 1) * HC], F16, name="staged_f")
            staged_b = cp.tile([H, (L + 1) * HC], F16, name="staged_b")
            nc.vector.memset(staged_f[:, 0:HC], 0.0)
            nc.vector.memset(staged_b[:, L * HC:(L + 1) * HC], 0.0)
            c_t = cp.tile([H, COLS], F32, name="c_t")
            nc.vector.memset(c_t[:], 0.0)
            hsout_f = cp.tile([H, EMIT * HC], F32, name="hsout_f")
            hsout_b = cp.tile([H, EMIT * HC], F32, name="hsout_b")

            # ---- stream phase ----
            with (
                tc.tile_pool(name="pwp", bufs=2, space="PSUM") as pwp,
                tc.tile_pool(name="prbp", bufs=1, space="PSUM") as prbp,
                tc.tile_pool(name="pgp", bufs=1, space="PSUM") as pgp,
            ):
                for r in range(RPC):
                    rb_ps = prbp.tile([128, W], F32, name="rb_ps")
                    nc.tensor.matmul(
                        rb_ps[:], ones_col[:], recip_sb[:, r * W:(r + 1) * W],
                        start=True, stop=True,
                    )
                    rb_sb = rbp.tile([128, W], F32, name="rb_sb")
                    nc.vector.tensor_copy(rb_sb[:], rb_ps[:])

                    s3s, As = [], []
                    for k in range(4):
                        ld = ldp.tile([128, 3 * D], F32, name="ld")
                        nc.sync.dma_start(
                            ld[:].rearrange("p (l d) -> p l d", l=3),
                            hid[:, r, 1 + k * 128: 1 + (k + 1) * 128, :]
                            .rearrange("l t d -> t l d"),
                        )
                        s3f_ = s3fp.tile([128, D], F32, name="s3f_")
                        nc.vector.tensor_add(s3f_[:], ld[:, 0:D], ld[:, D:2 * D])
                        s3_ = s3p.tile([128, D], F16, name="s3_")
                        nc.vector.tensor_add(s3_[:], s3f_[:], ld[:, 2 * D:3 * D])
                        A_ = apool.tile([128, W], F16, name="A_")
                        nc.vector.tensor_scalar(
                            A_[:], iota_f[:],
                            ids_f[:, r * 4 + k: r * 4 + k + 1], None, ALU.is_equal,
                        )
                        s3s.append(s3_)
                        As.append(A_)

                    pw = pwp.tile([128, 6 * W], F32, name="pw")
                    for m in range(6):
                        for k in range(4):
                            nc.tensor.matmul(
                                pw[:, m * W:(m + 1) * W],
                                s3s[k][:, m * 128:(m + 1) * 128], As[k][:],
                                start=(k == 0), stop=(k == 3),
                            )
                    xw = xwp.tile([128, 6 * W], F16, name="xw")
                    for m in range(6):
                        nc.vector.tensor_mul(
                            xw[:, m * W:(m + 1) * W], pw[:, m * W:(m + 1) * W], rb_sb[:]
                        )

                    pg = pgp.tile([GP, 2 * W], F32, name="pg")
                    for d in range(2):
                        for j in range(6):
                            nc.tensor.matmul(
                                pg[:, d * W:(d + 1) * W],
                                wih_sb[:, j * 2 * GP + d * GP: j * 2 * GP + (d + 1) * GP],
                                xw[:, j * W:(j + 1) * W],
                                start=(j == 0), stop=False,
                            )
                        nc.tensor.matmul(
                            pg[:, d * W:(d + 1) * W],
                            wih_sb[0:CAP, 6 * 2 * GP + d * GP: 6 * 2 * GP + (d + 1) * GP],
                            capT_sb[:, r * W:(r + 1) * W],
                            start=False, stop=True,
                        )
                    for d, base in ((0, GIN_F0), (1, GIN_B0)):
                        nc.scalar.activation(
                            _sb_ap(G_in, base + r, [[RPC, W]]),
                            pg[:, d * W:(d + 1) * W],
                            AF.Identity, bias=bias_sb[:, d:d + 1], scale=1.0,
                        )

            # ---- LSTM phase ----
            with (
                tc.tile_pool(name="plstm", bufs=2, space="PSUM") as plp,
                tc.tile_pool(name="gtp", bufs=2) as gtp,
                tc.tile_pool(name="atp", bufs=2) as atp,
                tc.tile_pool(name="tmpp", bufs=6) as tmpp,
            ):
                for t_ in range(L):
                    g_ps = plp.tile([GP, COLS], F32, name="g_ps")
                    nc.tensor.matmul(
                        g_ps[:, 0:HC], whh_sb[:, 0:GP],
                        staged_f[:, t_ * HC:(t_ + 1) * HC], start=True, stop=True,
                    )
                    nc.tensor.matmul(
                        g_ps[:, HC:COLS], whh_sb[:, GP:2 * GP],
                        staged_b[:, (L - t_) * HC:(L - t_ + 1) * HC],
                        start=True, stop=True,
                    )
                    g_t = gtp.tile([GP, COLS], F32, name="g_t")
                    dstep = GIN_B0 + (L - 1 - t_) * RPC - t_ * RPC
                    gin_slice = _sb_ap(
                        G_in, t_ * RPC, [[dstep, 2], [EMIT * RPC, S], [1, RPC]]
                    )
                    nc.vector.tensor_tensor(g_t[:], g_ps[:], gin_slice, ALU.add)

                    # per-gate tiles at base partition 0 (walrus: two-SB-input
                    # ops need equal base partitions; 1-input ACT can shift)
                    a_i = atp.tile([H, COLS], F32, name="a_i", tag="a_i")
                    nc.scalar.activation(a_i[:], g_t[GI:GI + H], AF.Sigmoid)
                    a_f = atp.tile([H, COLS], F32, name="a_f", tag="a_f")
                    nc.scalar.activation(a_f[:], g_t[GF:GF + H], AF.Sigmoid)
                    a_o = atp.tile([H, COLS], F32, name="a_o", tag="a_o")
                    nc.scalar.activation(a_o[:], g_t[GO:GO + H], AF.Sigmoid)
                    a_g = atp.tile([H, COLS], F32, name="a_g", tag="a_g")
                    nc.scalar.activation(a_g[:], g_t[GG:GG + H], AF.Tanh)

                    t1 = tmpp.tile([H, COLS], F32, name="t1")
                    nc.vector.tensor_mul(t1[:], a_i[:], a_g[:])
                    t2 = tmpp.tile([H, COLS], F32, name="t2")
                    nc.vector.tensor_mul(t2[:], a_f[:], c_t[:])
                    nc.vector.tensor_add(c_t[:], t1[:], t2[:])
                    tc_ = tmpp.tile([H, COLS], F32, name="tc_")
                    nc.scalar.activation(tc_[:], c_t[:], AF.Tanh)

                    nc.vector.tensor_mul(
                        staged_f[:, (t_ + 1) * HC:(t_ + 2) * HC],
                        a_o[:, 0:HC], tc_[:, 0:HC],
                    )
                    nc.vector.tensor_mul(
                        staged_b[:, (L - 1 - t_) * HC:(L - t_) * HC],
                        a_o[:, HC:COLS], tc_[:, HC:COLS],
                    )
                    if t_ >= WU:
                        # hsout cols = b*W + j*EMIT + tau'; src cols j*RPC+b
                        tp = t_ - WU
                        nc.vector.tensor_mul(
                            _sb_ap(hsout_f, tp, [[EMIT, S], [W, RPC]]),
                            a_o[:, 0:HC], tc_[:, 0:HC],
                        )
                        tpb = L - 1 - t_
                        nc.vector.tensor_mul(
                            _sb_ap(hsout_b, tpb, [[EMIT, S], [W, RPC]]),
                            a_o[:, HC:COLS], tc_[:, HC:COLS],
                        )

            # ---- output ----
            nc.sync.dma_start(
                bass.AP(out, 0, [[RPC * W, H], [1, RPC * W]]),
                _sb_ap(hsout_f, 0, [[1, RPC * W]]),
            )
            nc.sync.dma_start(
                bass.AP(out, H * RPC * W, [[RPC * W, H], [1, RPC * W]]),
                _sb_ap(hsout_b, 0, [[1, RPC * W]]),
            )

    if legalize:
        _legalize_waits(nc)
    nc.finalize()
    return nc


_NC_CACHE = {}


def get_nc():
    if "nc" not in _NC_CACHE:
        _NC_CACHE["nc"] = build_nc()
    return _NC_CACHE["nc"]


def host_prep(inputs):
    hiddens = np.asarray(inputs["hiddens"], dtype=np.float32)
    bert2toks = np.asarray(inputs["bert2toks"]).astype(np.int32)
    cap_inds = np.asarray(inputs["cap_inds"]).astype(np.int64)
    cap_table = np.asarray(inputs["cap_table"], dtype=np.float32)
    w_ih_f = np.asarray(inputs["w_ih_f"], dtype=np.float32)
    w_hh_f = np.asarray(inputs["w_hh_f"], dtype=np.float32)
    b_f = np.asarray(inputs["b_f"], dtype=np.float32)
    w_ih_b = np.asarray(inputs["w_ih_b"], dtype=np.float32)
    w_hh_b = np.asarray(inputs["w_hh_b"], dtype=np.float32)
    b_b = np.asarray(inputs["b_b"], dtype=np.float32)

    # gate rows i,f,g,o (pytorch) placed at 32-aligned starts i@0,f@32,o@64,g@96
    def pad_gates(w):  # [80, ...] -> [128, ...]
        out = np.zeros((GP,) + w.shape[1:], w.dtype)
        out[GI:GI + H] = w[0:H]
        out[GF:GF + H] = w[H:2 * H]
        out[GO:GO + H] = w[3 * H:4 * H]
        out[GG:GG + H] = w[2 * H:3 * H]
        return out

    wihT_np = np.zeros((INP_PAD, 2 * GP), np.float16)
    wihT_np[0:D + CAP, 0:GP] = pad_gates(w_ih_f).T
    wihT_np[0:D + CAP, GP:2 * GP] = pad_gates(w_ih_b).T
    whhT_np = np.concatenate(
        [pad_gates(w_hh_f).T, pad_gates(w_hh_b).T], axis=1
    ).astype(np.float16)
    bias_np = np.stack([pad_gates(b_f), pad_gates(b_b)], axis=1).astype(np.float32)

    cnt = np.zeros((B, W), np.float32)
    for bb in range(B):
        cnt[bb] = np.bincount(bert2toks[bb], minlength=W)[:W]
    with np.errstate(divide="ignore"):
        recip3_np = (1.0 / (3.0 * cnt)).astype(np.float32)

    capT_np = cap_table[cap_inds].transpose(0, 2, 1).astype(np.float16)  # [B,CAP,W]

    in_maps = []
    for c in range(NCORES):
        sl = slice(c * RPC, (c + 1) * RPC)
        in_maps.append({
            "hid": np.ascontiguousarray(hiddens[:, sl]),
            "wihT": wihT_np,
            "whhT": whhT_np,
            "bias": bias_np,
            "ids": np.ascontiguousarray(bert2toks[sl]),
            "recip3": np.ascontiguousarray(recip3_np[sl]),
            "capT": np.ascontiguousarray(capT_np[sl]),
        })
    return in_maps


def kernel(**inputs) -> np.ndarray:
    nc = get_nc()
    in_maps = host_prep(inputs)
    res = run_bass_kernel_spmd(nc, in_maps, core_ids=list(range(NCORES)))
    return assemble([r["out"] for r in res.results])


def assemble(outs):
    # each out: [2, H, RPC, W] -> [RPC, W, 2H]
    return np.concatenate(
        [np.asarray(o).transpose(2, 3, 0, 1).reshape(RPC, W, 2 * H) for o in outs],
        axis=0,
    ).astype(np.float32)


# revision 17
# speedup vs baseline: 1.7090x; 1.7090x over previous
"""Trainium2 Bass kernel for nn_BaseModel_20925080666480.

Pipeline per core (8 batch rows): 3-layer mean + ragged segment-mean via
one-hot matmul + cap-embedding concat + input projection, then a
segment-parallel BiLSTM (16 segments/dir, 28-step warmup) over 256 words.
"""

import numpy as np

import concourse.bass as bass
import concourse.tile as tile
from concourse import mybir
from concourse.bass_utils import run_bass_kernel_spmd

AF = mybir.ActivationFunctionType
ALU = mybir.AluOpType
F32 = mybir.dt.float32
F16 = mybir.dt.float16
I32 = mybir.dt.int32

NCORES = 8
B = 64
RPC = B // NCORES          # batch rows per core
T, W, D, H = 512, 256, 768, 20
GP = 128                   # padded gate rows (32-aligned starts)
GI, GF, GO, GG = 0, 32, 64, 96
CAP = 10
INP_PAD = 896              # 778 padded to 7*128
S = 16                     # segments per direction
EMIT = W // S              # 16 words emitted per segment
WU = 28                    # warmup steps
L = EMIT + WU              # 44 chain steps
HC = S * RPC               # 128 columns per direction
COLS = 2 * HC              # 256 total lstm columns
GIN_F0 = WU * RPC          # fwd zero-pad columns
GIN_B0 = GIN_F0 + W * RPC  # bwd data block start
GIN_SZ = 2 * (W * RPC + WU * RPC)
SBLK = (L + 1) * HC        # staged block size per direction (columns)
HOB = EMIT * HC            # hsout per-direction columns


def _sb_ap(t, offset, dims, prows=None):
    """Raw AP on a tile's tensor: partition pair from the tile (optionally
    overridden), custom free dims."""
    p = t.ap[0]
    pr = list(p) if prows is None else list(prows)
    return bass.AP(t.tensor, offset, [pr, *[list(x) for x in dims]])


def _legalize_waits(nc):
    """This walrus build accepts only ONE sync-wait per instruction; hoist
    extra waits onto NoOps inserted just before, on the same engine."""
    n = 0
    for fn in nc.m.functions:
        for b in fn.blocks:
            insts = b.instructions
            i = 0
            while i < len(insts):
                inst = insts[i]
                si = inst.sync_info
                if si is not None and len(si.on_wait) > 1:
                    waits = list(si.on_wait)
                    for w in waits[:-1]:
                        nop = mybir.InstNoOp(
                            name=f"wnop_{n}", engine=inst.engine, ins=[], outs=[]
                        )
                        nop.sync_info = mybir.SyncInfo(on_wait=[w], on_update=[])
                        insts.insert(i, nop)
                        i += 1
                        n += 1
                    inst.sync_info = mybir.SyncInfo(
                        on_wait=[waits[-1]], on_update=list(si.on_update)
                    )
                i += 1
    return n


def build_nc(legalize=True):
    nc = bass.Bass()
    hid = nc.dram_tensor("hid", [3, RPC, T + 1, D], F16, kind="ExternalInput")
    wihT = nc.dram_tensor("wihT", [INP_PAD, 2 * GP], F16, kind="ExternalInput")
    whhT = nc.dram_tensor("whhT", [H, 2 * GP], F16, kind="ExternalInput")
    bias = nc.dram_tensor("bias", [GP, 2], F32, kind="ExternalInput")
    ids = nc.dram_tensor("ids", [RPC, T], I32, kind="ExternalInput")
    recip3 = nc.dram_tensor("recip3", [RPC, W], F32, kind="ExternalInput")
    capT = nc.dram_tensor("capT", [RPC, CAP, W], F16, kind="ExternalInput")
    # [dir, h, b, w] fp16 — host transposes to [b, w, 2H] f32
    out = nc.dram_tensor("out", [2, H, RPC, W], F16, kind="ExternalOutput")

    with tile.TileContext(nc) as tc:
        with (
            tc.tile_pool(name="const", bufs=1) as cp,
            tc.tile_pool(name="ldp", bufs=4) as ldp,
            tc.tile_pool(name="s3fp", bufs=3) as s3fp,
            tc.tile_pool(name="s3p", bufs=6) as s3p,
            tc.tile_pool(name="Apool", bufs=6) as apool,
            tc.tile_pool(name="xwp", bufs=2) as xwp,
            tc.tile_pool(name="rbp", bufs=2) as rbp,
        ):
            # ---- constants ----
            iota_i = cp.tile([128, W], I32, name="iota_i")
            eng_iota = nc.gpsimd if hasattr(nc.gpsimd, "iota") else nc.vector
            eng_iota.iota(iota_i[:], pattern=[[1, W]], base=0, channel_multiplier=0)
            iota_h = cp.tile([128, W], F16, name="iota_h")
            nc.vector.tensor_copy(iota_h[:], iota_i[:])

            wih_sb = cp.tile([128, 7 * 2 * GP], F16, name="wih_sb")
            nc.sync.dma_start(
                wih_sb[:].rearrange("p (j g) -> p j g", j=7),
                wihT[:].rearrange("(j p) g -> p j g", p=128),
            )
            whh_sb = cp.tile([H, 2 * GP], F16, name="whh_sb")
            nc.sync.dma_start(whh_sb[:], whhT[:])
            # lhsT copy at base partition GO (matmul rhs lives at base GO)
            whh64 = cp.tile([GO + H, 2 * GP], F16, name="whh64")
            nc.vector.tensor_copy(whh64[GO:GO + H], whh_sb[:])
            bias_sb = cp.tile([GP, 2], F32, name="bias_sb")
            nc.sync.dma_start(bias_sb[:], bias[:])

            ids_sb = cp.tile([128, RPC * 4], I32, name="ids_sb")
            nc.sync.dma_start(
                ids_sb[:].rearrange("p (r k) -> p r k", r=RPC),
                ids[:].rearrange("r (k p) -> p r k", p=128),
            )
            ids_f = cp.tile([128, RPC * 4], F32, name="ids_f")
            nc.vector.tensor_copy(ids_f[:], ids_sb[:])

            recip_sb = cp.tile([1, RPC * W], F32, name="recip_sb")
            nc.sync.dma_start(
                recip_sb[:], recip3[:].rearrange("r w -> (r w)").unsqueeze(0)
            )
            capT_sb = cp.tile([CAP, RPC * W], F16, name="capT_sb")
            nc.sync.dma_start(
                capT_sb[:].rearrange("p (r w) -> p r w", r=RPC),
                capT[:].rearrange("r c w -> c r w"),
            )
            ones_col = cp.tile([1, 128], F32, name="ones_col")
            nc.vector.memset(ones_col[:], 1.0)

            # ---- persistent state ----
            G_in = cp.tile([GP, GIN_SZ], F16, name="G_in")
            nc.vector.memset(G_in[:], 0.0)
            # staged h (fp16): fwd block cols [0, SBLK), bwd block [SBLK, 2*SBLK)
            # rows GO:GO+H so the h-write TT shares base partition with o/tanh_c
            staged = cp.tile([GO + H, 2 * SBLK], F16, name="staged")
            nc.vector.memset(staged[GO:GO + H, 0:HC], 0.0)
            nc.vector.memset(staged[GO:GO + H, SBLK + L * HC:SBLK + (L + 1) * HC], 0.0)
            c_t = cp.tile([GF + H, COLS], F16, name="c_t")
            nc.vector.memset(c_t[GF:GF + H], 0.0)
            hsout = cp.tile([GO + H, 2 * HOB], F16, name="hsout")

            # ---- stream phase ----
            with (
                tc.tile_pool(name="pwp", bufs=2, space="PSUM") as pwp,
                tc.tile_pool(name="prbp", bufs=1, space="PSUM") as prbp,
                tc.tile_pool(name="pgp", bufs=1, space="PSUM") as pgp,
            ):
                for pr in range(RPC // 2):
                    xw = xwp.tile([128, 2 * 6 * W], F16, name="xw")
                    for half in range(2):
                        r = 2 * pr + half
                        rb_ps = prbp.tile([128, W], F32, name="rb_ps")
                        nc.tensor.matmul(
                            rb_ps[:], ones_col[:], recip_sb[:, r * W:(r + 1) * W],
                            start=True, stop=True,
                        )
                        rb_sb = rbp.tile([128, W], F32, name="rb_sb")
                        nc.vector.tensor_copy(rb_sb[:], rb_ps[:])

                        s3s, As = [], []
                        for k in range(4):
                            ld = ldp.tile([128, 3 * D], F16, name="ld")
                            nc.sync.dma_start(
                                ld[:].rearrange("p (l d) -> p l d", l=3),
                                hid[:, r, 1 + k * 128: 1 + (k + 1) * 128, :]
                                .rearrange("l t d -> t l d"),
                            )
                            s3f_ = s3fp.tile([128, D], F16, name="s3f_")
                            nc.vector.tensor_add(s3f_[:], ld[:, 0:D], ld[:, D:2 * D])
                            s3_ = s3p.tile([128, D], F16, name="s3_")
                            nc.vector.tensor_add(s3_[:], s3f_[:], ld[:, 2 * D:3 * D])
                            A_ = apool.tile([128, W], F16, name="A_")
                            nc.vector.tensor_scalar(
                                A_[:], iota_h[:],
                                ids_f[:, r * 4 + k: r * 4 + k + 1], None, ALU.is_equal,
                            )
                            s3s.append(s3_)
                            As.append(A_)

                        pw = pwp.tile([128, 6 * W], F32, name="pw")
                        for m in range(6):
                            for k in range(4):
                                nc.tensor.matmul(
                                    pw[:, m * W:(m + 1) * W],
                                    s3s[k][:, m * 128:(m + 1) * 128], As[k][:],
                                    start=(k == 0), stop=(k == 3),
                                )
                        for m in range(6):
                            nc.vector.tensor_mul(
                                xw[:, half * 6 * W + m * W: half * 6 * W + (m + 1) * W],
                                pw[:, m * W:(m + 1) * W], rb_sb[:],
                            )

                    # input projection for the row pair: rhs N=512 (2 rows)
                    pg = pgp.tile([GP, 2 * W], F32, name="pg")
                    for d_ in range(2):
                        for j in range(6):
                            rhs = _sb_ap(xw, j * W, [[6 * W, 2], [1, W]])
                            nc.tensor.matmul(
                                pg[:],
                                wih_sb[:, j * 2 * GP + d_ * GP: j * 2 * GP + (d_ + 1) * GP],
                                rhs, start=(j == 0), stop=False,
                            )
                        caprhs = _sb_ap(
                            capT_sb, 2 * pr * W, [[W, 2], [1, W]], prows=[capT_sb.ap[0][0], CAP]
                        )
                        nc.tensor.matmul(
                            pg[:],
                            wih_sb[0:CAP, 6 * 2 * GP + d_ * GP: 6 * 2 * GP + (d_ + 1) * GP],
                            caprhs, start=False, stop=True,
                        )
                        base = GIN_F0 if d_ == 0 else GIN_B0
                        nc.scalar.activation(
                            _sb_ap(G_in, base + 2 * pr, [[1, 2], [RPC, W]]),
                            pg[:],
                            AF.Identity, bias=bias_sb[:, d_:d_ + 1], scale=1.0,
                        )

            # ---- LSTM phase ----
            with (
                tc.tile_pool(name="plstm", bufs=3, space="PSUM") as plp,
                tc.tile_pool(name="gate", bufs=2) as gp_,
                tc.tile_pool(name="tmpp", bufs=6) as tmpp,
            ):
                for t_ in range(L):
                    # preload G_in slice into PSUM; matmuls accumulate on top
                    g_ps = plp.tile([GP, COLS], F32, name="g_ps")
                    dstep = GIN_B0 + (L - 1 - t_) * RPC - t_ * RPC
                    gin_slice = _sb_ap(
                        G_in, t_ * RPC, [[dstep, 2], [EMIT * RPC, S], [1, RPC]]
                    )
                    nc.vector.tensor_copy(g_ps[:], gin_slice)
                    nc.tensor.matmul(
                        g_ps[:, 0:HC], whh64[GO:GO + H, 0:GP],
                        staged[GO:GO + H, t_ * HC:(t_ + 1) * HC],
                        start=False, stop=True, skip_group_check=True,
                    )
                    nc.tensor.matmul(
                        g_ps[:, HC:COLS], whh64[GO:GO + H, GP:2 * GP],
                        staged[GO:GO + H, SBLK + (L - t_) * HC:SBLK + (L - t_ + 1) * HC],
                        start=False, stop=True, skip_group_check=True,
                    )
                    # gates (fp16): one sigmoid over rows 0:84 covers i,f,o
                    # (spans from partition 0 are unrestricted; non-zero
                    # starts are limited to 32 partitions). i is rebased to
                    # GF by a shifted 1-input DVE copy; tanh(g) lands at GF.
                    a_s = gp_.tile([GO + H, COLS], F16, name="a_s", tag="a_s")
                    nc.scalar.activation(a_s[0:GO + H], g_ps[0:GO + H], AF.Sigmoid)
                    i_t = tmpp.tile([GF + H, COLS], F16, name="i_t", tag="i_t")
                    nc.vector.tensor_copy(i_t[GF:GF + H], a_s[GI:GI + H])
                    gg = tmpp.tile([GF + H, COLS], F16, name="gg", tag="gg")
                    nc.scalar.activation(gg[GF:GF + H], g_ps[GG:GG + H], AF.Tanh)

                    t1 = tmpp.tile([GF + H, COLS], F16, name="t1", tag="t1")
                    nc.vector.tensor_mul(t1[GF:GF + H], i_t[GF:GF + H], gg[GF:GF + H])
                    t2 = tmpp.tile([GF + H, COLS], F16, name="t2", tag="t2")
                    nc.vector.tensor_mul(t2[GF:GF + H], a_s[GF:GF + H], c_t[GF:GF + H])
                    nc.vector.tensor_add(c_t[GF:GF + H], t1[GF:GF + H], t2[GF:GF + H])
                    tc_ = tmpp.tile([GO + H, COLS], F16, name="tc_", tag="tc_")
                    nc.scalar.activation(tc_[GO:GO + H], c_t[GF:GF + H], AF.Tanh)

                    # h write: one TT into both staged blocks (fwd @ (t+1),
                    # bwd @ SBLK + (L-1-t)); cols 0:HC fwd | HC:COLS bwd
                    hstep = (SBLK + (L - 1 - t_) * HC) - (t_ + 1) * HC
                    hdst = bass.AP(
                        staged.tensor,
                        GO * staged.ap[0][0] + (t_ + 1) * HC,
                        [[staged.ap[0][0], H], [hstep, 2], [1, HC]],
                    )
                    nc.vector.tensor_mul(hdst, a_s[GO:GO + H], tc_[GO:GO + H])
                    if t_ >= WU:
                        # hsout cols = b*W + j*EMIT + tau' (w contiguous per b)
                        tp = t_ - WU
                        pitch = hsout.ap[0][0]
                        nc.vector.tensor_mul(
                            bass.AP(hsout.tensor, GO * pitch + tp,
                                    [[pitch, H], [EMIT, S], [W, RPC]]),
                            a_s[GO:GO + H, 0:HC], tc_[GO:GO + H, 0:HC],
                        )
                        tpb = L - 1 - t_
                        nc.vector.tensor_mul(
                            bass.AP(hsout.tensor, GO * pitch + HOB + tpb,
                                    [[pitch, H], [EMIT, S], [W, RPC]]),
                            a_s[GO:GO + H, HC:COLS], tc_[GO:GO + H, HC:COLS],
                        )

            # ---- output (fp16; host casts) ----
            pitch = hsout.ap[0][0]
            nc.sync.dma_start(
                bass.AP(out, 0, [[RPC * W, H], [1, RPC * W]]),
                bass.AP(hsout.tensor, GO * pitch, [[pitch, H], [1, RPC * W]]),
            )
            nc.sync.dma_start(
                bass.AP(out, H * RPC * W, [[RPC * W, H], [1, RPC * W]]),
                bass.AP(hsout.tensor, GO * pitch + HOB, [[pitch, H], [1, RPC * W]]),
            )

    if legalize:
        _legalize_waits(nc)
    nc.finalize()
    return nc


_NC_CACHE = {}


def get_nc():
    if "nc" not in _NC_CACHE:
        _NC_CACHE["nc"] = build_nc()
    return _NC_CACHE["nc"]


def host_prep(inputs):
    hiddens = np.asarray(inputs["hiddens"], dtype=np.float32)
    bert2toks = np.asarray(inputs["bert2toks"]).astype(np.int32)
    cap_inds = np.asarray(inputs["cap_inds"]).astype(np.int64)
    cap_table = np.asarray(inputs["cap_table"], dtype=np.float32)
    w_ih_f = np.asarray(inputs["w_ih_f"], dtype=np.float32)
    w_hh_f = np.asarray(inputs["w_hh_f"], dtype=np.float32)
    b_f = np.asarray(inputs["b_f"], dtype=np.float32)
    w_ih_b = np.asarray(inputs["w_ih_b"], dtype=np.float32)
    w_hh_b = np.asarray(inputs["w_hh_b"], dtype=np.float32)
    b_b = np.asarray(inputs["b_b"], dtype=np.float32)

    # gate rows i,f,g,o (pytorch) placed at 32-aligned starts i@0,f@32,o@64,g@96
    def pad_gates(w):  # [80, ...] -> [128, ...]
        out = np.zeros((GP,) + w.shape[1:], w.dtype)
        out[GI:GI + H] = w[0:H]
        out[GF:GF + H] = w[H:2 * H]
        out[GO:GO + H] = w[3 * H:4 * H]
        out[GG:GG + H] = w[2 * H:3 * H]
        return out

    wihT_np = np.zeros((INP_PAD, 2 * GP), np.float16)
    wihT_np[0:D + CAP, 0:GP] = pad_gates(w_ih_f).T
    wihT_np[0:D + CAP, GP:2 * GP] = pad_gates(w_ih_b).T
    whhT_np = np.concatenate(
        [pad_gates(w_hh_f).T, pad_gates(w_hh_b).T], axis=1
    ).astype(np.float16)
    bias_np = np.stack([pad_gates(b_f), pad_gates(b_b)], axis=1).astype(np.float32)

    cnt = np.zeros((B, W), np.float32)
    for bb in range(B):
        cnt[bb] = np.bincount(bert2toks[bb], minlength=W)[:W]
    with np.errstate(divide="ignore"):
        recip3_np = (1.0 / (3.0 * cnt)).astype(np.float32)

    capT_np = cap_table[cap_inds].transpose(0, 2, 1).astype(np.float16)  # [B,CAP,W]
    hid16 = hiddens.astype(np.float16)

    in_maps = []
    for c in range(NCORES):
        sl = slice(c * RPC, (c + 1) * RPC)
        in_maps.append({
            "hid": np.ascontiguousarray(hid16[:, sl]),
            "wihT": wihT_np,
            "whhT": whhT_np,
            "bias": bias_np,
            "ids": np.ascontiguousarray(bert2toks[sl]),
            "recip3": np.ascontiguousarray(recip3_np[sl]),
            "capT": np.ascontiguousarray(capT_np[sl]),
        })
    return in_maps


def kernel(**inputs) -> np.ndarray:
    nc = get_nc()
    in_maps = host_prep(inputs)
    res = run_bass_kernel_spmd(nc, in_maps, core_ids=list(range(NCORES)))
    return assemble([r["out"] for r in res.results])


def assemble(outs):
    # each out: [2, H, RPC, W] -> [RPC, W, 2H]
    return np.concatenate(
        [
            np.asarray(o).astype(np.float32)
            .transpose(2, 3, 0, 1).reshape(RPC, W, 2 * H)
            for o in outs
        ],
        axis=0,
    ).astype(np.float32)
